# revision 6
# baseline (speedup 1.0000x reference)
"""Distributed brute-force kNN (retrieval) on 8 TRN2 NeuronCores.

reference semantics:
    dist[b,m] = ||q_b||^2 + ||p_m||^2 - 2 q_b.p_m        # [256, 200000]
    nn_idx = top_k(-dist, 16)                            # [256, 16]
    out = trajectories[nn_idx]                           # [256, 16, 8, 3]

Strategy (bank sharded over M across 8 cores):
  - per core: negdist = (2q) @ bankT_shard - bank_sq  via PE fp32 matmuls
    (queries stationary [128k,128q], bank moving [128k,512m]; bank_sq folded
    in as a K=1 ones-broadcast matmul into the same PSUM accumulation).
    ||q||^2 is constant per query row and cannot change the top-k order.
  - per core local top-16 per 8704-wide chunk via DVE max8/match_replace/
    max_index (3 chunks -> 48 candidates/query/core).
  - AllGather candidates (value + global-index-as-f32 packed) -> 384/query.
  - every core reduces 384 -> global top-16 (identical result), selects its
    2 rank slots (per-core ranksel input), resolves position->global index
    with an is_equal-mask + multiply + reduce (no per-partition gather op
    exists), then indirect-DMA-gathers trajectory rows from its full copy.
  - host interleaves the 8 cores' [256, 2, 8, 3] rank-slot outputs.
"""

import sys

sys.path.insert(0, "/opt/trn_rl_repo")

import numpy as np

import concourse.bacc as bacc
import concourse.bass as bass
import concourse.mybir as mybir
import concourse.tile as tile
from concourse.bass_utils import run_bass_kernel_spmd

f32 = mybir.dt.float32
i32 = mybir.dt.int32
u32 = mybir.dt.uint32

P = 128          # partitions / queries per block
QB = 2           # query blocks (256 queries)
C = 1024         # feature dim
KT = C // P      # 8 contraction tiles
M = 200000
NCORES = 8
MS = M // NCORES          # 25000 real m per core
MT = 512                  # psum tile width (one fp32 PSUM bank)
NT = 49                   # m-tiles per core (49*512 = 25088 >= 25000)
MP = NT * MT              # 25088 padded m per core
CHUNKS = [8704, 8704, 7680]   # scan chunks (17+17+15 tiles = 49)
NCH = len(CHUNKS)
KC = 16                   # candidates per chunk
NCAND = NCH * KC          # 48 per core per query
NALL = NCORES * NCAND     # 384 gathered candidates
NEG = -1.0e30


def build_kernel():
    nc = bacc.Bacc(None)
    q2T_d = nc.declare_dram_parameter("q2T", [C, QB * P], f32, isOutput=False)
    bankT_d = nc.declare_dram_parameter("bankT", [C, MP], f32, isOutput=False)
    nbsq_d = nc.declare_dram_parameter("nbsq", [1, MP], f32, isOutput=False)
    ones_d = nc.declare_dram_parameter("ones", [1, P], f32, isOutput=False)
    ioff_d = nc.declare_dram_parameter("ioff", [P, NCAND], f32, isOutput=False)
    rsel_d = nc.declare_dram_parameter("rsel", [P, 2], f32, isOutput=False)
    iota16_d = nc.declare_dram_parameter("iota16", [P, 16], f32, isOutput=False)
    iotaN_d = nc.declare_dram_parameter("iotaN", [P, NALL], f32, isOutput=False)
    traj_d = nc.declare_dram_parameter("traj", [M, 24], f32, isOutput=False)
    out_d = nc.declare_dram_parameter("out", [QB, P, 2, 24], f32, isOutput=True)

    with tile.TileContext(nc) as tc:
        with (
            tc.tile_pool(name="const", bufs=1) as const,
            tc.tile_pool(name="bankp", bufs=4) as bankp,
            tc.tile_pool(name="nbsqp", bufs=4) as nbsqp,
            tc.tile_pool(name="slabp", bufs=2) as slabp,
            tc.tile_pool(name="psp", bufs=2, space="PSUM") as psp,
            tc.tile_pool(name="candp", bufs=1) as candp,
            tc.tile_pool(name="finp", bufs=2) as finp,
            tc.tile_pool(name="drp", bufs=1, space="DRAM") as drp,
        ):
            # ---- constants ----
            qts = []
            for k in range(KT):
                qt = const.tile([P, QB * P], f32, name=f"qt{k}")
                nc.sync.dma_start(qt[:], q2T_d[k * P:(k + 1) * P, :])
                qts.append(qt)
            ones_t = const.tile([1, P], f32, name="ones_t")
            nc.sync.dma_start(ones_t[:], ones_d[:])
            ioff_t = const.tile([P, NCAND], f32, name="ioff_t")
            nc.sync.dma_start(ioff_t[:], ioff_d[:])
            rsel_t = const.tile([P, 2], f32, name="rsel_t")
            nc.sync.dma_start(rsel_t[:], rsel_d[:])
            iota16_t = const.tile([P, 16], f32, name="iota16_t")
            nc.sync.dma_start(iota16_t[:], iota16_d[:])
            iotaN_t = const.tile([P, NALL], f32, name="iotaN_t")
            nc.sync.dma_start(iotaN_t[:], iotaN_d[:])

            cv = [candp.tile([P, NCAND], f32, name=f"cv{qb}") for qb in range(QB)]
            cpos = [candp.tile([P, NCAND], u32, name=f"cpos{qb}") for qb in range(QB)]

            # ---- main loop: matmul into psum, copy to slab, scan per chunk ----
            gt = 0  # global m-tile index
            for ch in range(NCH):
                cw = CHUNKS[ch]
                ctiles = cw // MT
                slabs = [
                    slabp.tile([P, 8704], f32, tag=f"slab{qb}", name=f"slab{qb}_{ch}")
                    for qb in range(QB)
                ]
                for tl in range(ctiles):
                    m0 = gt * MT
                    psts = [
                        psp.tile([P, MT], f32, tag=f"ps{qb}", name=f"ps{qb}_{gt}")
                        for qb in range(QB)
                    ]
                    nb = nbsqp.tile([1, MT], f32, tag="nb", name=f"nb{gt}")
                    nc.sync.dma_start(nb[:], nbsq_d[0:1, m0:m0 + MT])
                    for qb in range(QB):
                        nc.tensor.matmul(out=psts[qb][:], lhsT=ones_t[:], rhs=nb[:],
                                         start=True, stop=False)
                    for k in range(KT):
                        bk = bankp.tile([P, MT], f32, tag="bank", name=f"bk{gt}_{k}")
                        nc.sync.dma_start(bk[:], bankT_d[k * P:(k + 1) * P, m0:m0 + MT])
                        for qb in range(QB):
                            nc.tensor.matmul(
                                out=psts[qb][:],
                                lhsT=qts[k][:, qb * P:(qb + 1) * P],
                                rhs=bk[:],
                                start=False, stop=(k == KT - 1),
                            )
                    for qb in range(QB):
                        nc.scalar.copy(slabs[qb][:, tl * MT:(tl + 1) * MT], psts[qb][:])
                    gt += 1
                # scans: top-16 of this chunk per query block
                for qb in range(QB):
                    s = slabs[qb][:, 0:cw]
                    c0 = ch * KC
                    nc.vector.max(cv[qb][:, c0:c0 + 8], s)
                    nc.vector.max_index(cpos[qb][:, c0:c0 + 8], cv[qb][:, c0:c0 + 8], s)
                    nc.vector.match_replace(s, in_to_replace=cv[qb][:, c0:c0 + 8],
                                            in_values=s, imm_value=NEG)
                    nc.vector.max(cv[qb][:, c0 + 8:c0 + 16], s)
                    nc.vector.max_index(cpos[qb][:, c0 + 8:c0 + 16],
                                        cv[qb][:, c0 + 8:c0 + 16], s)

            # ---- candidate global indices as f32; pack (vals | idx) ----
            cl = drp.tile([QB, P, 2 * NCAND], f32, name="cl")
            ag = drp.tile([NCORES * QB, P, 2 * NCAND], f32, addr_space="Shared",
                          name="ag")
            for qb in range(QB):
                cposf = candp.tile([P, NCAND], f32, name=f"cposf{qb}")
                nc.vector.tensor_copy(cposf[:], cpos[qb][:])
                cidx = candp.tile([P, NCAND], f32, name=f"cidx{qb}")
                nc.vector.tensor_tensor(out=cidx[:], in0=cposf[:], in1=ioff_t[:],
                                        op=mybir.AluOpType.add)
                nc.sync.dma_start(cl[qb, :, 0:NCAND], cv[qb][:])
                nc.sync.dma_start(cl[qb, :, NCAND:2 * NCAND], cidx[:])
            nc.gpsimd.collective_compute(
                "AllGather", mybir.AluOpType.bypass,
                replica_groups=[list(range(NCORES))],
                ins=[cl[:]], outs=[ag[:]],
            )

            # ---- final reduce (identical on all cores) + per-core rank slots ----
            for qb in range(QB):
                av = finp.tile([P, NALL], f32, tag="av", name=f"av{qb}")
                ai = finp.tile([P, NALL], f32, tag="ai", name=f"ai{qb}")
                for cc in range(NCORES):
                    nc.sync.dma_start(av[:, cc * NCAND:(cc + 1) * NCAND],
                                      ag[cc * QB + qb, :, 0:NCAND])
                    nc.sync.dma_start(ai[:, cc * NCAND:(cc + 1) * NCAND],
                                      ag[cc * QB + qb, :, NCAND:2 * NCAND])
                fv = finp.tile([P, 16], f32, tag="fv", name=f"fv{qb}")
                fpos = finp.tile([P, 16], u32, tag="fpos", name=f"fpos{qb}")
                nc.vector.max(fv[:, 0:8], av[:])
                nc.vector.max_index(fpos[:, 0:8], fv[:, 0:8], av[:])
                nc.vector.match_replace(av[:], in_to_replace=fv[:, 0:8],
                                        in_values=av[:], imm_value=NEG)
                nc.vector.max(fv[:, 8:16], av[:])
                nc.vector.max_index(fpos[:, 8:16], fv[:, 8:16], av[:])
                fposf = finp.tile([P, 16], f32, tag="fposf", name=f"fposf{qb}")
                nc.vector.tensor_copy(fposf[:], fpos[:])
                for r in range(2):
                    # rank slot for this core: j = rsel[:, r]; myfpos = fposf[j]
                    m16 = finp.tile([P, 16], f32, tag="m16", name=f"m16{qb}{r}")
                    nc.vector.tensor_tensor(
                        out=m16[:], in0=iota16_t[:],
                        in1=rsel_t[:, r:r + 1].to_broadcast([P, 16]),
                        op=mybir.AluOpType.is_equal)
                    nc.vector.tensor_tensor(out=m16[:], in0=m16[:], in1=fposf[:],
                                            op=mybir.AluOpType.mult)
                    myfpos = finp.tile([P, 1], f32, tag="myfpos", name=f"myfpos{qb}{r}")
                    nc.vector.tensor_reduce(myfpos[:], m16[:],
                                            mybir.AxisListType.X,
                                            mybir.AluOpType.add)
                    # global index = ai[myfpos]
                    mN = finp.tile([P, NALL], f32, tag="mN", name=f"mN{qb}{r}")
                    nc.vector.tensor_tensor(
                        out=mN[:], in0=iotaN_t[:],
                        in1=myfpos[:, 0:1].to_broadcast([P, NALL]),
                        op=mybir.AluOpType.is_equal)
                    nc.vector.tensor_tensor(out=mN[:], in0=mN[:], in1=ai[:],
                                            op=mybir.AluOpType.mult)
                    gidxf = finp.tile([P, 1], f32, tag="gidxf", name=f"gidxf{qb}{r}")
                    nc.vector.tensor_reduce(gidxf[:], mN[:],
                                            mybir.AxisListType.X,
                                            mybir.AluOpType.add)
                    gidx = finp.tile([P, 1], i32, tag="gidx", name=f"gidx{qb}{r}")
                    nc.vector.tensor_copy(gidx[:], gidxf[:])
                    trg = finp.tile([P, 24], f32, tag="trg", name=f"trg{qb}{r}")
                    nc.gpsimd.indirect_dma_start(
                        out=trg[:], out_offset=None,
                        in_=traj_d[:],
                        in_offset=bass.IndirectOffsetOnAxis(ap=gidx[:, 0:1], axis=0),
                    )
                    nc.sync.dma_start(out_d[qb, :, r, :], trg[:])
    return nc


_CACHED = {}


def _prepare_inputs(query, bank, trajectories):
    query = np.asarray(query, dtype=np.float32)
    bank = np.asarray(bank, dtype=np.float32)
    traj = np.ascontiguousarray(
        np.asarray(trajectories, dtype=np.float32).reshape(M, 24))
    q2T = np.ascontiguousarray((2.0 * query).T)                    # [1024, 256]
    bsq64 = np.einsum("mc,mc->m", bank.astype(np.float64), bank.astype(np.float64))
    nbsq_full = (-bsq64).astype(np.float32)
    ones = np.ones((1, P), np.float32)
    iota16 = np.broadcast_to(np.arange(16, dtype=np.float32), (P, 16)).copy()
    iotaN = np.broadcast_to(np.arange(NALL, dtype=np.float32), (P, NALL)).copy()
    choff = np.zeros(NCAND, np.float32)
    for ch in range(NCH):
        choff[ch * KC:(ch + 1) * KC] = sum(CHUNKS[:ch])

    in_maps = []
    for c in range(NCORES):
        sl = slice(c * MS, (c + 1) * MS)
        bankT = np.zeros((C, MP), np.float32)
        bankT[:, :MS] = bank[sl].T
        nbsq = np.full((1, MP), NEG, np.float32)
        nbsq[0, :MS] = nbsq_full[sl]
        ioff = np.broadcast_to(choff + np.float32(c * MS), (P, NCAND)).astype(
            np.float32)
        rsel = np.broadcast_to(
            np.array([2 * c, 2 * c + 1], np.float32), (P, 2)).copy()
        in_maps.append({
            "q2T": q2T, "bankT": bankT, "nbsq": nbsq, "ones": ones,
            "ioff": np.ascontiguousarray(ioff), "rsel": rsel,
            "iota16": iota16, "iotaN": iotaN, "traj": traj,
        })
    return in_maps


def _assemble(results):
    out = np.empty((QB * P, 16, 8, 3), np.float32)
    for c in range(NCORES):
        o = results[c]["out"].reshape(QB * P, 2, 8, 3)
        out[:, 2 * c] = o[:, 0]
        out[:, 2 * c + 1] = o[:, 1]
    return out


def _run(in_maps, trace=False):
    if "nc" not in _CACHED:
        nc = build_kernel()
        nc.compile()
        _CACHED["nc"] = nc
    nc = _CACHED["nc"]
    res = run_bass_kernel_spmd(nc, in_maps, core_ids=list(range(NCORES)),
                               trace=trace)
    return res


def kernel(query, bank, trajectories, k):
    assert int(k) == 16, f"kernel hardcodes k=16, got {k}"
    assert query.shape == (QB * P, C) and bank.shape == (M, C)
    in_maps = _prepare_inputs(query, bank, trajectories)
    res = _run(in_maps, trace=False)
    return _assemble(res.results)


if __name__ == "__main__":
    # smoke build
    build_kernel()
    print("build ok")


# revision 14
# speedup vs baseline: 1.2556x; 1.2556x over previous
"""Distributed brute-force kNN (retrieval) on 8 TRN2 NeuronCores.

reference semantics:
    dist[b,m] = ||q_b||^2 + ||p_m||^2 - 2 q_b.p_m        # [256, 200000]
    nn_idx = top_k(-dist, 16)                            # [256, 16]
    out = trajectories[nn_idx]                           # [256, 16, 8, 3]

Strategy (bank sharded over M across 8 cores):
  - per core: negdist = (2q) @ bankT_shard - bank_sq  via PE fp32 matmuls
    (queries stationary [128k,128q], bank moving [128k,512m]; bank_sq folded
    in as a K=1 ones-broadcast matmul into the same PSUM accumulation).
    ||q||^2 is constant per query row and cannot change the top-k order.
  - per core local top-16 per 8704-wide chunk via DVE max8/match_replace/
    max_index (3 chunks -> 48 candidates/query/core).
  - AllGather candidates (value + global-index-as-f32 packed) -> 384/query.
  - every core reduces 384 -> global top-16 (identical result), selects its
    2 rank slots (per-core ranksel input), resolves position->global index
    with an is_equal-mask + multiply + reduce (no per-partition gather op
    exists), then indirect-DMA-gathers trajectory rows from its full copy.
  - host interleaves the 8 cores' [256, 2, 8, 3] rank-slot outputs.
"""

import sys

sys.path.insert(0, "/opt/trn_rl_repo")

import ml_dtypes
import numpy as np

import concourse.bacc as bacc
import concourse.bass as bass
import concourse.mybir as mybir
import concourse.tile as tile
from concourse.bass_utils import run_bass_kernel_spmd

f32 = mybir.dt.float32
bf16 = mybir.dt.bfloat16
i32 = mybir.dt.int32
u32 = mybir.dt.uint32
np_bf16 = ml_dtypes.bfloat16

P = 128          # partitions / queries per block
QB = 2           # query blocks (256 queries)
C = 1024         # feature dim
KT = C // P      # 8 contraction tiles
M = 200000
NCORES = 8
MS = M // NCORES          # 25000 real m per core
MT = 512                  # psum tile width (one fp32 PSUM bank)
NT = 49                   # m-tiles per core (49*512 = 25088 >= 25000)
MP = NT * MT              # 25088 padded m per core
CHUNKS = [8704, 8704, 7680]   # scan chunks (17+17+15 tiles = 49)
NCH = len(CHUNKS)
KC = 16                   # candidates per chunk
NCAND = NCH * KC          # 48 per core per query
NALL = NCORES * NCAND     # 384 gathered candidates
NEG = -1.0e30


def build_kernel():
    nc = bacc.Bacc(None)
    qhT_d = nc.declare_dram_parameter("qhT", [C, QB * P], bf16, isOutput=False)
    qlT_d = nc.declare_dram_parameter("qlT", [C, QB * P], bf16, isOutput=False)
    bankTh_d = nc.declare_dram_parameter("bankTh", [C, MP], bf16, isOutput=False)
    bankTl_d = nc.declare_dram_parameter("bankTl", [C, MP], bf16, isOutput=False)
    nbsq_d = nc.declare_dram_parameter("nbsq", [1, MP], f32, isOutput=False)
    ones_d = nc.declare_dram_parameter("ones", [1, P], f32, isOutput=False)
    ioff_d = nc.declare_dram_parameter("ioff", [P, NCAND], f32, isOutput=False)
    rsel_d = nc.declare_dram_parameter("rsel", [P, 2], f32, isOutput=False)
    iota16_d = nc.declare_dram_parameter("iota16", [P, 16], f32, isOutput=False)
    iotaN_d = nc.declare_dram_parameter("iotaN", [P, NALL], f32, isOutput=False)
    traj_d = nc.declare_dram_parameter("traj", [M, 24], f32, isOutput=False)
    out_d = nc.declare_dram_parameter("out", [QB, P, 2, 24], f32, isOutput=True)

    with tile.TileContext(nc) as tc:
        with (
            tc.tile_pool(name="const", bufs=1) as const,
            tc.tile_pool(name="bankp", bufs=4) as bankp,
            tc.tile_pool(name="nbsqp", bufs=4) as nbsqp,
            tc.tile_pool(name="slabp", bufs=2) as slabp,
            tc.tile_pool(name="psp", bufs=2, space="PSUM") as psp,
            tc.tile_pool(name="candp", bufs=1) as candp,
            tc.tile_pool(name="finp", bufs=2) as finp,
            tc.tile_pool(name="drp", bufs=1, space="DRAM") as drp,
        ):
            # ---- constants ----
            qhs, qls = [], []
            for k in range(KT):
                qht = const.tile([P, QB * P], bf16, name=f"qht{k}")
                nc.sync.dma_start(qht[:], qhT_d[k * P:(k + 1) * P, :])
                qhs.append(qht)
                qlt = const.tile([P, QB * P], bf16, name=f"qlt{k}")
                nc.sync.dma_start(qlt[:], qlT_d[k * P:(k + 1) * P, :])
                qls.append(qlt)
            ones_t = const.tile([1, P], f32, name="ones_t")
            nc.sync.dma_start(ones_t[:], ones_d[:])
            ioff_t = const.tile([P, NCAND], f32, name="ioff_t")
            nc.sync.dma_start(ioff_t[:], ioff_d[:])
            rsel_t = const.tile([P, 2], f32, name="rsel_t")
            nc.sync.dma_start(rsel_t[:], rsel_d[:])
            iota16_t = const.tile([P, 16], f32, name="iota16_t")
            nc.sync.dma_start(iota16_t[:], iota16_d[:])
            iotaN_t = const.tile([P, NALL], f32, name="iotaN_t")
            nc.sync.dma_start(iotaN_t[:], iotaN_d[:])

            cv = [candp.tile([P, NCAND], f32, name=f"cv{qb}") for qb in range(QB)]
            cpos = [candp.tile([P, NCAND], u32, name=f"cpos{qb}") for qb in range(QB)]

            # ---- main loop: matmul into psum, copy to slab, scan per chunk ----
            gt = 0  # global m-tile index
            for ch in range(NCH):
                cw = CHUNKS[ch]
                ctiles = cw // MT
                slabs = [
                    slabp.tile([P, 8704], f32, tag=f"slab{qb}", name=f"slab{qb}_{ch}")
                    for qb in range(QB)
                ]
                for tl in range(ctiles):
                    m0 = gt * MT
                    psts = [
                        psp.tile([P, MT], f32, tag=f"ps{qb}", name=f"ps{qb}_{gt}")
                        for qb in range(QB)
                    ]
                    nb = nbsqp.tile([1, MT], f32, tag="nb", name=f"nb{gt}")
                    nc.sync.dma_start(nb[:], nbsq_d[0:1, m0:m0 + MT])
                    for qb in range(QB):
                        nc.tensor.matmul(out=psts[qb][:], lhsT=ones_t[:], rhs=nb[:],
                                         start=True, stop=False)
                    for k in range(KT):
                        bkh = bankp.tile([P, MT], bf16, tag="bankh", name=f"bkh{gt}_{k}")
                        nc.sync.dma_start(bkh[:],
                                          bankTh_d[k * P:(k + 1) * P, m0:m0 + MT])
                        bkl = bankp.tile([P, MT], bf16, tag="bankl", name=f"bkl{gt}_{k}")
                        nc.sync.dma_start(bkl[:],
                                          bankTl_d[k * P:(k + 1) * P, m0:m0 + MT])
                        last = (k == KT - 1)
                        for qb in range(QB):
                            qh_sl = qhs[k][:, qb * P:(qb + 1) * P]
                            ql_sl = qls[k][:, qb * P:(qb + 1) * P]
                            nc.tensor.matmul(out=psts[qb][:], lhsT=qh_sl, rhs=bkh[:],
                                             start=False, stop=False)
                            nc.tensor.matmul(out=psts[qb][:], lhsT=qh_sl, rhs=bkl[:],
                                             start=False, stop=False)
                            nc.tensor.matmul(out=psts[qb][:], lhsT=ql_sl, rhs=bkh[:],
                                             start=False, stop=last)
                    for qb in range(QB):
                        nc.scalar.copy(slabs[qb][:, tl * MT:(tl + 1) * MT], psts[qb][:])
                    gt += 1
                # scans: top-16 of this chunk per query block
                for qb in range(QB):
                    s = slabs[qb][:, 0:cw]
                    c0 = ch * KC
                    nc.vector.max(cv[qb][:, c0:c0 + 8], s)
                    nc.vector.max_index(cpos[qb][:, c0:c0 + 8], cv[qb][:, c0:c0 + 8], s)
                    nc.vector.match_replace(s, in_to_replace=cv[qb][:, c0:c0 + 8],
                                            in_values=s, imm_value=NEG)
                    nc.vector.max(cv[qb][:, c0 + 8:c0 + 16], s)
                    nc.vector.max_index(cpos[qb][:, c0 + 8:c0 + 16],
                                        cv[qb][:, c0 + 8:c0 + 16], s)

            # ---- candidate global indices as f32; pack (vals | idx) ----
            cl = drp.tile([QB, P, 2 * NCAND], f32, name="cl")
            ag = drp.tile([NCORES * QB, P, 2 * NCAND], f32, addr_space="Shared",
                          name="ag")
            for qb in range(QB):
                cposf = candp.tile([P, NCAND], f32, name=f"cposf{qb}")
                nc.vector.tensor_copy(cposf[:], cpos[qb][:])
                cidx = candp.tile([P, NCAND], f32, name=f"cidx{qb}")
                nc.vector.tensor_tensor(out=cidx[:], in0=cposf[:], in1=ioff_t[:],
                                        op=mybir.AluOpType.add)
                nc.sync.dma_start(cl[qb, :, 0:NCAND], cv[qb][:])
                nc.sync.dma_start(cl[qb, :, NCAND:2 * NCAND], cidx[:])
            nc.gpsimd.collective_compute(
                "AllGather", mybir.AluOpType.bypass,
                replica_groups=[list(range(NCORES))],
                ins=[cl[:]], outs=[ag[:]],
            )

            # ---- final reduce (identical on all cores) + per-core rank slots ----
            for qb in range(QB):
                av = finp.tile([P, NALL], f32, tag="av", name=f"av{qb}")
                ai = finp.tile([P, NALL], f32, tag="ai", name=f"ai{qb}")
                for cc in range(NCORES):
                    nc.sync.dma_start(av[:, cc * NCAND:(cc + 1) * NCAND],
                                      ag[cc * QB + qb, :, 0:NCAND])
                    nc.sync.dma_start(ai[:, cc * NCAND:(cc + 1) * NCAND],
                                      ag[cc * QB + qb, :, NCAND:2 * NCAND])
                fv = finp.tile([P, 16], f32, tag="fv", name=f"fv{qb}")
                fpos = finp.tile([P, 16], u32, tag="fpos", name=f"fpos{qb}")
                nc.vector.max(fv[:, 0:8], av[:])
                nc.vector.max_index(fpos[:, 0:8], fv[:, 0:8], av[:])
                nc.vector.match_replace(av[:], in_to_replace=fv[:, 0:8],
                                        in_values=av[:], imm_value=NEG)
                nc.vector.max(fv[:, 8:16], av[:])
                nc.vector.max_index(fpos[:, 8:16], fv[:, 8:16], av[:])
                fposf = finp.tile([P, 16], f32, tag="fposf", name=f"fposf{qb}")
                nc.vector.tensor_copy(fposf[:], fpos[:])
                for r in range(2):
                    # rank slot for this core: j = rsel[:, r]; myfpos = fposf[j]
                    m16 = finp.tile([P, 16], f32, tag="m16", name=f"m16{qb}{r}")
                    nc.vector.tensor_tensor(
                        out=m16[:], in0=iota16_t[:],
                        in1=rsel_t[:, r:r + 1].to_broadcast([P, 16]),
                        op=mybir.AluOpType.is_equal)
                    nc.vector.tensor_tensor(out=m16[:], in0=m16[:], in1=fposf[:],
                                            op=mybir.AluOpType.mult)
                    myfpos = finp.tile([P, 1], f32, tag="myfpos", name=f"myfpos{qb}{r}")
                    nc.vector.tensor_reduce(myfpos[:], m16[:],
                                            mybir.AxisListType.X,
                                            mybir.AluOpType.add)
                    # global index = ai[myfpos]
                    mN = finp.tile([P, NALL], f32, tag="mN", name=f"mN{qb}{r}")
                    nc.vector.tensor_tensor(
                        out=mN[:], in0=iotaN_t[:],
                        in1=myfpos[:, 0:1].to_broadcast([P, NALL]),
                        op=mybir.AluOpType.is_equal)
                    nc.vector.tensor_tensor(out=mN[:], in0=mN[:], in1=ai[:],
                                            op=mybir.AluOpType.mult)
                    gidxf = finp.tile([P, 1], f32, tag="gidxf", name=f"gidxf{qb}{r}")
                    nc.vector.tensor_reduce(gidxf[:], mN[:],
                                            mybir.AxisListType.X,
                                            mybir.AluOpType.add)
                    gidx = finp.tile([P, 1], i32, tag="gidx", name=f"gidx{qb}{r}")
                    nc.vector.tensor_copy(gidx[:], gidxf[:])
                    trg = finp.tile([P, 24], f32, tag="trg", name=f"trg{qb}{r}")
                    nc.gpsimd.indirect_dma_start(
                        out=trg[:], out_offset=None,
                        in_=traj_d[:],
                        in_offset=bass.IndirectOffsetOnAxis(ap=gidx[:, 0:1], axis=0),
                    )
                    nc.sync.dma_start(out_d[qb, :, r, :], trg[:])
    return nc


_CACHED = {}


def _prepare_inputs(query, bank, trajectories):
    query = np.asarray(query, dtype=np.float32)
    bank = np.asarray(bank, dtype=np.float32)
    traj = np.ascontiguousarray(
        np.asarray(trajectories, dtype=np.float32).reshape(M, 24))
    q2 = 2.0 * query
    qh = q2.astype(np_bf16)
    ql = (q2 - qh.astype(np.float32)).astype(np_bf16)
    qhT = np.ascontiguousarray(qh.T)                               # [1024, 256] bf16
    qlT = np.ascontiguousarray(ql.T)
    bsq64 = np.einsum("mc,mc->m", bank.astype(np.float64), bank.astype(np.float64))
    nbsq_full = (-bsq64).astype(np.float32)
    ones = np.ones((1, P), np.float32)
    iota16 = np.broadcast_to(np.arange(16, dtype=np.float32), (P, 16)).copy()
    iotaN = np.broadcast_to(np.arange(NALL, dtype=np.float32), (P, NALL)).copy()
    choff = np.zeros(NCAND, np.float32)
    for ch in range(NCH):
        choff[ch * KC:(ch + 1) * KC] = sum(CHUNKS[:ch])

    in_maps = []
    for c in range(NCORES):
        sl = slice(c * MS, (c + 1) * MS)
        bsh = bank[sl].astype(np_bf16)
        bsl = (bank[sl] - bsh.astype(np.float32)).astype(np_bf16)
        bankTh = np.zeros((C, MP), np_bf16)
        bankTh[:, :MS] = bsh.T
        bankTl = np.zeros((C, MP), np_bf16)
        bankTl[:, :MS] = bsl.T
        nbsq = np.full((1, MP), NEG, np.float32)
        nbsq[0, :MS] = nbsq_full[sl]
        ioff = np.broadcast_to(choff + np.float32(c * MS), (P, NCAND)).astype(
            np.float32)
        rsel = np.broadcast_to(
            np.array([2 * c, 2 * c + 1], np.float32), (P, 2)).copy()
        in_maps.append({
            "qhT": qhT, "qlT": qlT, "bankTh": bankTh, "bankTl": bankTl,
            "nbsq": nbsq, "ones": ones,
            "ioff": np.ascontiguousarray(ioff), "rsel": rsel,
            "iota16": iota16, "iotaN": iotaN, "traj": traj,
        })
    return in_maps


def _assemble(results):
    out = np.empty((QB * P, 16, 8, 3), np.float32)
    for c in range(NCORES):
        o = results[c]["out"].reshape(QB * P, 2, 8, 3)
        out[:, 2 * c] = o[:, 0]
        out[:, 2 * c + 1] = o[:, 1]
    return out


def _run(in_maps, trace=False):
    if "nc" not in _CACHED:
        nc = build_kernel()
        nc.compile()
        _CACHED["nc"] = nc
    nc = _CACHED["nc"]
    res = run_bass_kernel_spmd(nc, in_maps, core_ids=list(range(NCORES)),
                               trace=trace)
    return res


def kernel(query, bank, trajectories, k):
    assert int(k) == 16, f"kernel hardcodes k=16, got {k}"
    assert query.shape == (QB * P, C) and bank.shape == (M, C)
    in_maps = _prepare_inputs(query, bank, trajectories)
    res = _run(in_maps, trace=False)
    return _assemble(res.results)


if __name__ == "__main__":
    # smoke build
    build_kernel()
    print("build ok")
